# revision 12
# baseline (speedup 1.0000x reference)
"""Trainium2 Bass kernel for nn_Causal_model_vae (MoE-routed VAE).

Reference computation (N=16384 tokens, DX=DH=1024, S=8 experts):
    mu_h     = leaky(data @ Wm1 + bm1) @ Wm2 + bm2
    logvar_h = leaky(data @ Wv1 + bv1) @ Wv2 + bv2
    h_sample = eps * exp(0.5*logvar_h) + mu_h
    reconstruct[n] = (leaky(h_sample @ We1[s_n] + be1[s_n]) @ We2[s_n] + be2[s_n])
returns (reconstruct, mu_h, logvar_h, h_sample).

Strategy: perfectly load-balanced expert-parallelism.  The routing ids `s`
are known on the host, so the host sorts tokens by expert and cuts the
sorted order into 8 chunks of exactly N/8 = 2048 tokens -- every core does
identical work (the bf16 PE roofline is 6 matmul layers x 2048 tokens).
A chunk's minority tokens (those not of its majority expert -- <=0.6% for
balanced random s) are decoded on-device with the WRONG (majority) weights
to keep the SPMD program uniform; the host recomputes just their decoder
from the device-computed h_sample and overwrites those rows.

On-chip layout: everything is feature-major [feature, token] so chained
matmuls need no transposes; matmul operands are bf16 (f32 PSUM
accumulation), the sampling chain and all outputs are f32.  All DRAM
tensors use the flat SBUF-image layout [128, KT*width] so every DMA is
fully contiguous per partition.  Weight fetches ride the gpsimd queue in
parallel with the sync queue's x/eps fetches so the first matmul starts
~4us earlier; the aux We2 image reuses wm1's SBUF buffer (dead after the
last encoder block) to stay inside SBUF.

The decoder is software-pipelined one token-block behind the encoder so
the PE never idles while the sampling chain (ACT exp + DVE fma) drains.

Biases are structurally zero in this problem's setup_inputs(); the kernel
asserts that and skips them on-device.  Any violated structural
assumption falls back to an exact host computation.
"""

import contextlib
import ctypes
import os
import sys
import types

import numpy as np
import ml_dtypes

import concourse.bacc as bacc
import concourse.bass as bass
import concourse.mybir as mybir
import concourse.tile as tile
from concourse.bass_utils import run_bass_kernel_spmd

N, DX, DH, S = 16384, 1024, 1024, 8
KT = DH // 128    # 8 k-tiles (DX == DH == 1024)
SLOPE = 0.01
NCORES = 8
T = 256           # token block width (matmul moving dim)
C = N // NCORES   # 2048 tokens per core, exactly balanced

BF16 = mybir.dt.bfloat16
F32 = mybir.dt.float32

LAST_RESULTS = None  # BassKernelResults of the most recent run (for profiling)

_program_cache: dict[str, "bacc.Bacc"] = {}


def _ensure_ntff_hook():
    """bass_utils imports antenv.axon_hooks when tracing under axon; some
    images lack that module.  Install a ctypes-based equivalent if so."""
    try:
        import antenv.axon_hooks  # noqa: F401
        return
    except ImportError:
        pass
    try:
        import antenv

        so_path = "/opt/axon/libaxon_pjrt.so"
        if not os.path.exists(so_path):
            return
        lib = ctypes.CDLL(so_path)
        if not hasattr(lib, "axon_start_nrt_profile"):
            return
        lib.axon_start_nrt_profile.argtypes = [
            ctypes.POINTER(ctypes.c_int64), ctypes.c_size_t]
        lib.axon_start_nrt_profile.restype = ctypes.c_int64
        lib.axon_stop_nrt_profile.argtypes = [ctypes.c_char_p]
        lib.axon_stop_nrt_profile.restype = ctypes.c_int64

        @contextlib.contextmanager
        def _hook(output_dir, device_ids):
            import jax

            jax.devices()
            if device_ids:
                ids = (ctypes.c_int64 * len(device_ids))(*device_ids)
                rc = lib.axon_start_nrt_profile(ids, len(device_ids))
            else:
                rc = lib.axon_start_nrt_profile(None, 0)
            if rc != 0:
                raise RuntimeError(f"axon_start_nrt_profile rc={rc}")
            try:
                yield
            finally:
                n = lib.axon_stop_nrt_profile(str(output_dir).encode())
                print(f"ntff profile: {n} file(s) -> {output_dir}")

        m = types.ModuleType("antenv.axon_hooks")
        m.get_axon_ntff_profile_hook = lambda: _hook
        m.set_axon_ntff_profile_hook = lambda h: None
        sys.modules["antenv.axon_hooks"] = m
        antenv.axon_hooks = m
    except Exception:
        pass


BLOCKS = [(i * T, T) for i in range(C // T)]   # 8 x 256
NBLOCKS = len(BLOCKS)


def build_program() -> "bacc.Bacc":
    nc = bacc.Bacc("TRN2", target_bir_lowering=False, debug=False,
                   num_devices=NCORES)

    xT = nc.dram_tensor("xT", [128, KT * C], BF16, kind="ExternalInput").ap()
    epsT = nc.dram_tensor("epsT", [128, KT * C], BF16,
                          kind="ExternalInput").ap()
    wnames = ["wm1", "wv1", "wm2", "wv2", "we1", "we2"]
    wdram = {n: nc.dram_tensor(n, [128, KT * 1024], BF16,
                               kind="ExternalInput").ap() for n in wnames}
    outs = {n: nc.dram_tensor(n, [128, KT * C], F32,
                              kind="ExternalOutput").ap()
            for n in ["muT", "lvT", "hT", "recT"]}

    Exp = mybir.ActivationFunctionType.Exp
    Copy = mybir.ActivationFunctionType.Copy
    mult = mybir.AluOpType.mult
    max_ = mybir.AluOpType.max
    add = mybir.AluOpType.add

    with tile.TileContext(nc) as tc:
        with (
            tc.tile_pool(name="wpool", bufs=1) as wpool,
            tc.tile_pool(name="io2", bufs=2) as io2,
            tc.tile_pool(name="io", bufs=1) as io,
            tc.tile_pool(name="iost", bufs=2) as iost,
            tc.tile_pool(name="mid", bufs=1) as mid,
            # Separate PSUM pools so the (one-block-delayed) decoder's
            # slot-recycling waits never reference encoder matmul progress.
            tc.tile_pool(name="psum_e", bufs=5,
                         space=bass.MemorySpace.PSUM) as psum_e,
            tc.tile_pool(name="psum_d", bufs=3,
                         space=bass.MemorySpace.PSUM) as psum_d,
        ):
            xt_tiles = {}
            eps_tiles = {}

            def fetch_block(b, x_only=False):
                if b >= NBLOCKS:
                    return
                off, w = BLOCKS[b]
                if b not in xt_tiles:
                    x = io2.tile([128, KT * w], BF16, tag="x")
                    nc.sync.dma_start(x[:], xT[:, off * KT : off * KT + KT * w])
                    xt_tiles[b] = x
                if not x_only and b not in eps_tiles:
                    e = io2.tile([128, KT * w], BF16, tag="eps")
                    nc.sync.dma_start(e[:], epsT[:, off * KT : off * KT + KT * w])
                    eps_tiles[b] = e

            wt = {}

            QW = KT * 256  # one dout-quarter of a weight image

            def fetch_weight(name, eng, quarters=1):
                w = wpool.tile([128, KT * 1024], BF16,
                               tag=f"w_{name}", name=f"wt_{name}")
                step = 4 * QW // quarters
                for i in range(quarters):
                    eng.dma_start(w[:, i * step : (i + 1) * step],
                                  wdram[name][:, i * step : (i + 1) * step])
                wt[name] = w

            # PE warm-up: the first real matmul waits ~5us for x0+wm1.q0 DMA,
            # during which HAM throttles the PE to 1.2GHz.  Run
            # dependency-free dummy matmuls on a zeroed tile in that window
            # so the real stream starts at 2.4GHz (needs >=3.4us activity).
            # They cycle the DECODER psum ring (idle until ~t=34us) so the
            # encoder's first psum allocation never waits behind them.
            warm = io2.tile([128, 256], BF16, tag="warm")
            nc.vector.memset(warm[:], 0.0)
            for _ in range(16):
                ps_w = psum_d.tile([128, 512], F32, tag="ps")
                nc.tensor.matmul(ps_w[:, :256], warm[:, :128], warm[:],
                                 start=True, stop=True)

            # Input DMAs ride the sync queue; weights are split in usage
            # order across the gpsimd and scalar queues.  All three queues
            # progress concurrently, so the first real matmul needs only
            # x0 + wm1's first quarter, and each weight lands a few us
            # before the matmul stream reaches it.
            fetch_block(0, x_only=True)
            fetch_weight("wm1", nc.gpsimd, quarters=4)
            fetch_weight("wv1", nc.scalar, quarters=2)
            fetch_block(0)
            fetch_block(1)
            fetch_weight("wm2", nc.gpsimd)
            fetch_weight("wv2", nc.scalar)
            fetch_weight("we1", nc.gpsimd)
            fetch_weight("we2", nc.scalar)

            def layer(w, rhs, tw, out_cb, pool=None, first_after=None,
                      rhs_off=0, rhs_stride=None):
                """One 1024->1024 matmul layer on [128, tw]-sliced bf16 rhs.

                out_cb(mp, ps) consumes the [128, 2*tw] f32 psum of m-pair mp
                (psum tiles are always allocated [128, 512]; only the first
                2*tw columns are written).  rhs k-slice k starts at
                rhs_off + k*rhs_stride (rhs_stride defaults to tw; pass the
                enclosing block width T for sub-sliced decodes).
                first_after: ordering hint -- schedule this layer's first
                matmul after that instruction.
                """
                pool = pool or psum_e
                stride = rhs_stride if rhs_stride is not None else tw
                mm = None
                for mp in range(4):
                    ps = pool.tile([128, 512], F32, tag="ps")
                    for half in range(2):
                        m = 2 * mp + half
                        wo = (m // 2) * (KT * 256) + (m % 2) * 128
                        for k in range(KT):
                            ro = rhs_off + k * stride
                            mm = nc.tensor.matmul(
                                ps[:, half * tw : (half + 1) * tw],
                                w[:, wo + k * 256 : wo + k * 256 + 128],
                                rhs[:, ro : ro + tw],
                                start=(k == 0),
                                stop=(k == KT - 1),
                            )
                            if first_after is not None:
                                tile.add_dep_helper(
                                    mm.ins, first_after.ins, sync=False,
                                    reason="decoder pipelined behind next block")
                                first_after = None
                    out_cb(mp, ps)
                return mm

            def leaky_to(dst, tw, dst_off=0):
                def cb(mp, ps):
                    # leaky(x) = max(x, 0.01x); DVE can read PSUM only
                    # once per op, so stage 0.01x in SBUF first.
                    lk = io2.tile([128, 512], F32, tag="lk")
                    nc.vector.tensor_scalar_mul(
                        lk[:, : 2 * tw], ps[:, : 2 * tw], SLOPE)
                    nc.vector.tensor_tensor(
                        dst[:, dst_off + 2 * mp * tw :
                            dst_off + (2 * mp + 2) * tw],
                        lk[:, : 2 * tw], ps[:, : 2 * tw], max_)
                return cb

            def enc_block(b):
                """Encoder + sampling for block b; returns the bf16 h tile."""
                off, tw = BLOCKS[b]
                x, epst = xt_tiles.pop(b), eps_tiles.pop(b)
                fetch_block(b + 1)

                h1m = mid.tile([128, KT * tw], BF16, tag="h1m")
                l1m_last = layer(wt["wm1"], x, tw, leaky_to(h1m, tw))
                h1v = mid.tile([128, KT * tw], BF16, tag="h1v")
                layer(wt["wv1"], x, tw, leaky_to(h1v, tw))

                mu_f = io.tile([128, KT * tw], F32, tag="mu_f")

                def mu_cb(mp, ps):
                    nc.scalar.activation(
                        mu_f[:, 2 * mp * tw : (2 * mp + 2) * tw],
                        ps[:, : 2 * tw], Copy)

                layer(wt["wm2"], h1m, tw, mu_cb)
                nc.sync.dma_start(outs["muT"][:, off * KT : off * KT + KT * tw],
                                  mu_f[:])

                std_f = mid.tile([128, KT * tw], F32, tag="std_f")
                tmp_f = mid.tile([128, KT * tw], F32, tag="tmp_f")
                h_f = io.tile([128, KT * tw], F32, tag="h_f")
                h_b = io2.tile([128, KT * tw], BF16, tag="h_b")

                def lv_cb(mp, ps):
                    sl = slice(2 * mp * tw, (2 * mp + 2) * tw)
                    # lv staged per m-pair so the DMA drains early and the
                    # full-block f32 tile is saved.
                    lv_s = iost.tile([128, 512], F32, tag="lv_s")
                    nc.scalar.activation(lv_s[:, : 2 * tw], ps[:, : 2 * tw],
                                         Copy)
                    nc.sync.dma_start(
                        outs["lvT"][:, off * KT + 2 * mp * tw :
                                    off * KT + (2 * mp + 2) * tw],
                        lv_s[:, : 2 * tw])
                    nc.scalar.activation(std_f[:, sl], ps[:, : 2 * tw],
                                         Exp, scale=0.5)
                    # h = eps*std + mu, per m-pair so it pipelines
                    nc.vector.tensor_tensor(
                        tmp_f[:, sl], epst[:, sl], std_f[:, sl], mult)
                    nc.vector.tensor_tensor(
                        h_f[:, sl], tmp_f[:, sl], mu_f[:, sl], add)
                    nc.vector.tensor_tensor(
                        h_b[:, sl], tmp_f[:, sl], mu_f[:, sl], add)

                layer(wt["wv2"], h1v, tw, lv_cb)
                nc.sync.dma_start(outs["hT"][:, off * KT : off * KT + KT * tw],
                                  h_f[:])
                return h_b, l1m_last

            def dec_block(b, h_b, after=None):
                """Decoder (this core's main expert) for block b."""
                off, tw = BLOCKS[b]
                d1 = mid.tile([128, KT * tw], BF16, tag="d1")
                layer(wt["we1"], h_b, tw, leaky_to(d1, tw), pool=psum_d,
                      first_after=after)

                def rec_cb(mp, ps):
                    rec_s = iost.tile([128, 512], F32, tag="rec_s")
                    nc.scalar.activation(rec_s[:, : 2 * tw], ps[:, : 2 * tw],
                                         Copy)
                    # per-m-pair output DMA so the tail drains early
                    nc.sync.dma_start(
                        outs["recT"][:, off * KT + 2 * mp * tw :
                                     off * KT + (2 * mp + 2) * tw],
                        rec_s[:, : 2 * tw])

                layer(wt["we2"], d1, tw, rec_cb, pool=psum_d)

            # Software-pipeline the decoder one block behind the encoder:
            # while block b's sampling chain (ACT exp + DVE fma) drains,
            # the PE is busy on block b-1's decoder -- no PE idle at block
            # boundaries (which would also re-throttle the HAM clock).
            prev = None
            for b in range(NBLOCKS):
                h_b, l1m_last = enc_block(b)
                if prev is not None:
                    dec_block(b - 1, prev, after=l1m_last)
                prev = h_b
            dec_block(NBLOCKS - 1, prev)

    nc.compile()
    return nc


def _get_program() -> "bacc.Bacc":
    if "v2" not in _program_cache:
        _program_cache["v2"] = build_program()
    return _program_cache["v2"]


def _to_sbuf_image(arrT, width):
    """[1024, width] feature-major -> [128, KT*width] flat image, blockwise."""
    out = np.empty((128, KT * arrT.shape[1]), dtype=arrT.dtype)
    for off in range(0, arrT.shape[1], T):
        w = min(T, arrT.shape[1] - off)
        seg = arrT[:, off:off + w].reshape(KT, 128, w).transpose(1, 0, 2)
        out[:, off * KT : off * KT + KT * w] = seg.reshape(128, KT * w)
    return out


def _from_sbuf_image(img, width):
    """[128, KT*width] flat SBUF image -> [1024, width] feature-major."""
    out = np.empty((1024, width), dtype=img.dtype)
    for off in range(0, width, T):
        w = min(T, width - off)
        seg = img[:, off * KT : off * KT + KT * w].reshape(128, KT, w)
        out[:, off:off + w] = seg.transpose(1, 0, 2).reshape(1024, w)
    return out


def _weight_image(W):
    """[1024 din, 1024 dout] -> [128, KT*1024] flat lhsT image.

    dout-quarter-major: flat = (m//256)*(KT*256) + k*256 + (m%256), so each
    dout quarter is one contiguous per-partition range (the head can load
    wm1 quarter-by-quarter and start matmuls at 1/4 the critical DMA bytes).
    """
    img = W.reshape(KT, 128, 4, 256).transpose(1, 2, 0, 3)
    return np.ascontiguousarray(img.reshape(128, KT * 1024))


def _kernel_numpy(inputs):
    """Exact f32 fallback (used only if an assumption is violated)."""
    d = {k: np.asarray(v) for k, v in inputs.items()}
    leaky = lambda v: np.where(v > 0, v, np.float32(SLOPE) * v)
    mu = leaky(d["data"] @ d["Wm1"] + d["bm1"]) @ d["Wm2"] + d["bm2"]
    lv = leaky(d["data"] @ d["Wv1"] + d["bv1"]) @ d["Wv2"] + d["bv2"]
    h = d["eps"] * np.exp(0.5 * lv) + mu
    s = np.asarray(d["s"]).astype(np.int64)
    rec = np.empty_like(d["data"])
    for e in range(d["We1"].shape[0]):
        m = s == e
        rec[m] = leaky(h[m] @ d["We1"][e] + d["be1"][e]) @ d["We2"][e] + d["be2"][e]
    return rec, mu, lv, h


def _partition_tokens(s):
    """Cut the expert-sorted token order into NCORES chunks of C tokens.

    Every chunk is decoded on-device with its MAJORITY expert's weights;
    the few minority ("rest") tokens at the chunk edges get their decoder
    recomputed on the host (0.6% of tokens for balanced random s) from the
    device-computed h_sample.  Returns per-core (ids, main_expert) where
    ids is ordered main-tokens-first.
    """
    order = np.argsort(s, kind="stable")
    cores = []
    for c in range(NCORES):
        chunk = order[c * C : (c + 1) * C]
        experts = s[chunk]
        uniq, counts = np.unique(experts, return_counts=True)
        main_e = int(uniq[np.argmax(counts)])
        is_main = experts == main_e
        ids = np.concatenate([chunk[is_main], chunk[~is_main]])
        cores.append((ids, main_e))
    return cores


def kernel(**inputs) -> tuple:
    data = np.ascontiguousarray(np.asarray(inputs["data"], dtype=np.float32))
    eps = np.ascontiguousarray(np.asarray(inputs["eps"], dtype=np.float32))
    s = np.asarray(inputs["s"]).astype(np.int64)
    # The device kernel folds the (structurally zero) biases away; any
    # violated assumption falls back to an exact host computation.
    nonzero_bias = any(
        np.abs(np.asarray(inputs[b])).max() != 0.0
        for b in ("bm1", "bm2", "bv1", "bv2", "be1", "be2"))
    if nonzero_bias or data.shape != (N, DX) or s.shape != (N,):
        return _kernel_numpy(inputs)
    cores = _partition_tokens(s)

    nc = _get_program()
    bf = ml_dtypes.bfloat16

    wimg = {
        "wm1": _weight_image(np.asarray(inputs["Wm1"], np.float32)).astype(bf),
        "wm2": _weight_image(np.asarray(inputs["Wm2"], np.float32)).astype(bf),
        "wv1": _weight_image(np.asarray(inputs["Wv1"], np.float32)).astype(bf),
        "wv2": _weight_image(np.asarray(inputs["Wv2"], np.float32)).astype(bf),
    }
    We1 = np.asarray(inputs["We1"], np.float32)
    We2 = np.asarray(inputs["We2"], np.float32)
    dec_img = {}

    def dec_images(e):
        if e not in dec_img:
            dec_img[e] = (_weight_image(We1[e]).astype(bf),
                          _weight_image(We2[e]).astype(bf))
        return dec_img[e]

    dataT = data.T
    epsT = eps.T

    in_maps = []
    for ids, main_e in cores:
        we1, we2 = dec_images(main_e)
        in_maps.append({
            "xT": _to_sbuf_image(
                np.ascontiguousarray(dataT[:, ids]).astype(bf), C),
            "epsT": _to_sbuf_image(
                np.ascontiguousarray(epsT[:, ids]).astype(bf), C),
            "wm1": wimg["wm1"], "wm2": wimg["wm2"],
            "wv1": wimg["wv1"], "wv2": wimg["wv2"],
            "we1": we1, "we2": we2,
        })

    global LAST_RESULTS
    _ensure_ntff_hook()
    res = run_bass_kernel_spmd(nc, in_maps, list(range(NCORES)))
    LAST_RESULTS = res

    mu = np.empty((N, DH), np.float32)
    lv = np.empty((N, DH), np.float32)
    h = np.empty((N, DH), np.float32)
    rec = np.empty((N, DX), np.float32)
    for c, (ids, main_e) in enumerate(cores):
        r = res.results[c]
        mu[ids] = _from_sbuf_image(r["muT"], C).T
        lv[ids] = _from_sbuf_image(r["lvT"], C).T
        h[ids] = _from_sbuf_image(r["hT"], C).T
        rec[ids] = _from_sbuf_image(r["recT"], C).T

    # Host-side decoder for the minority tokens of each chunk (<=0.6% of
    # tokens): their on-device decode used the majority expert's weights.
    leaky = lambda v: np.where(v > 0, v, np.float32(SLOPE) * v)
    rest = np.concatenate([ids[s[ids] != main_e] for ids, main_e in cores])
    if rest.size:
        for e in np.unique(s[rest]):
            m = rest[s[rest] == e]
            rec[m] = leaky(h[m] @ We1[e]) @ We2[e]
    return rec, mu, lv, h


# revision 16
# speedup vs baseline: 1.0091x; 1.0091x over previous
"""Trainium2 Bass kernel for nn_Causal_model_vae (MoE-routed VAE).

Reference computation (N=16384 tokens, DX=DH=1024, S=8 experts):
    mu_h     = leaky(data @ Wm1 + bm1) @ Wm2 + bm2
    logvar_h = leaky(data @ Wv1 + bv1) @ Wv2 + bv2
    h_sample = eps * exp(0.5*logvar_h) + mu_h
    reconstruct[n] = (leaky(h_sample @ We1[s_n] + be1[s_n]) @ We2[s_n] + be2[s_n])
returns (reconstruct, mu_h, logvar_h, h_sample).

Strategy: perfectly load-balanced expert-parallelism.  The routing ids `s`
are known on the host, so the host sorts tokens by expert and cuts the
sorted order into 8 chunks of exactly N/8 = 2048 tokens -- every core does
identical work (the bf16 PE roofline is 6 matmul layers x 2048 tokens).
A chunk's minority tokens (those not of its majority expert -- <=0.6% for
balanced random s) are decoded on-device with the WRONG (majority) weights
to keep the SPMD program uniform; the host recomputes just their decoder
from the device-computed h_sample and overwrites those rows.

On-chip layout: everything is feature-major [feature, token] so chained
matmuls need no transposes; matmul operands are bf16 (f32 PSUM
accumulation), the sampling chain and all outputs are f32.  All DRAM
tensors use the flat SBUF-image layout [128, KT*width] so every DMA is
fully contiguous per partition.  Weight fetches ride the gpsimd queue in
parallel with the sync queue's x/eps fetches so the first matmul starts
~4us earlier; the aux We2 image reuses wm1's SBUF buffer (dead after the
last encoder block) to stay inside SBUF.

The decoder is software-pipelined one token-block behind the encoder so
the PE never idles while the sampling chain (ACT exp + DVE fma) drains.

Biases are structurally zero in this problem's setup_inputs(); the kernel
asserts that and skips them on-device.  Any violated structural
assumption falls back to an exact host computation.
"""

import contextlib
import ctypes
import os
import sys
import types

import numpy as np
import ml_dtypes

import concourse.bacc as bacc
import concourse.bass as bass
import concourse.mybir as mybir
import concourse.tile as tile
from concourse.bass_utils import run_bass_kernel_spmd

N, DX, DH, S = 16384, 1024, 1024, 8
KT = DH // 128    # 8 k-tiles (DX == DH == 1024)
SLOPE = 0.01
NCORES = 8
T = 256           # token block width (matmul moving dim)
C = N // NCORES   # 2048 tokens per core, exactly balanced

BF16 = mybir.dt.bfloat16
F32 = mybir.dt.float32

LAST_RESULTS = None  # BassKernelResults of the most recent run (for profiling)

_program_cache: dict[str, "bacc.Bacc"] = {}


def _ensure_ntff_hook():
    """bass_utils imports antenv.axon_hooks when tracing under axon; some
    images lack that module.  Install a ctypes-based equivalent if so."""
    try:
        import antenv.axon_hooks  # noqa: F401
        return
    except ImportError:
        pass
    try:
        import antenv

        so_path = "/opt/axon/libaxon_pjrt.so"
        if not os.path.exists(so_path):
            return
        lib = ctypes.CDLL(so_path)
        if not hasattr(lib, "axon_start_nrt_profile"):
            return
        lib.axon_start_nrt_profile.argtypes = [
            ctypes.POINTER(ctypes.c_int64), ctypes.c_size_t]
        lib.axon_start_nrt_profile.restype = ctypes.c_int64
        lib.axon_stop_nrt_profile.argtypes = [ctypes.c_char_p]
        lib.axon_stop_nrt_profile.restype = ctypes.c_int64

        @contextlib.contextmanager
        def _hook(output_dir, device_ids):
            import jax

            jax.devices()
            if device_ids:
                ids = (ctypes.c_int64 * len(device_ids))(*device_ids)
                rc = lib.axon_start_nrt_profile(ids, len(device_ids))
            else:
                rc = lib.axon_start_nrt_profile(None, 0)
            if rc != 0:
                raise RuntimeError(f"axon_start_nrt_profile rc={rc}")
            try:
                yield
            finally:
                n = lib.axon_stop_nrt_profile(str(output_dir).encode())
                print(f"ntff profile: {n} file(s) -> {output_dir}")

        m = types.ModuleType("antenv.axon_hooks")
        m.get_axon_ntff_profile_hook = lambda: _hook
        m.set_axon_ntff_profile_hook = lambda h: None
        sys.modules["antenv.axon_hooks"] = m
        antenv.axon_hooks = m
    except Exception:
        pass


BLOCKS = [(i * T, T) for i in range(C // T)]   # 8 x 256
NBLOCKS = len(BLOCKS)


def build_program() -> "bacc.Bacc":
    nc = bacc.Bacc("TRN2", target_bir_lowering=False, debug=False,
                   num_devices=NCORES)

    xT = nc.dram_tensor("xT", [128, KT * C], BF16, kind="ExternalInput").ap()
    epsT = nc.dram_tensor("epsT", [128, KT * C], BF16,
                          kind="ExternalInput").ap()
    wnames = ["wm1", "wv1", "wm2", "wv2", "we1", "we2"]
    wdram = {n: nc.dram_tensor(n, [128, KT * 1024], BF16,
                               kind="ExternalInput").ap() for n in wnames}
    outs = {n: nc.dram_tensor(n, [128, KT * C], F32,
                              kind="ExternalOutput").ap()
            for n in ["muT", "lvT", "hT", "recT"]}

    Exp = mybir.ActivationFunctionType.Exp
    Copy = mybir.ActivationFunctionType.Copy
    mult = mybir.AluOpType.mult
    max_ = mybir.AluOpType.max
    add = mybir.AluOpType.add

    with tile.TileContext(nc) as tc:
        with (
            tc.tile_pool(name="wpool", bufs=1) as wpool,
            tc.tile_pool(name="io2", bufs=2) as io2,
            tc.tile_pool(name="io", bufs=1) as io,
            tc.tile_pool(name="iost", bufs=2) as iost,
            tc.tile_pool(name="mid", bufs=1) as mid,
            # Separate PSUM pools so the (one-block-delayed) decoder's
            # slot-recycling waits never reference encoder matmul progress.
            tc.tile_pool(name="psum_e", bufs=5,
                         space=bass.MemorySpace.PSUM) as psum_e,
            tc.tile_pool(name="psum_d", bufs=3,
                         space=bass.MemorySpace.PSUM) as psum_d,
        ):
            xt_tiles = {}
            eps_tiles = {}

            def fetch_block(b, x_only=False):
                if b >= NBLOCKS:
                    return
                off, w = BLOCKS[b]
                if b not in xt_tiles:
                    x = io2.tile([128, KT * w], BF16, tag="x")
                    nc.sync.dma_start(x[:], xT[:, off * KT : off * KT + KT * w])
                    xt_tiles[b] = x
                if not x_only and b not in eps_tiles:
                    e = io2.tile([128, KT * w], BF16, tag="eps")
                    nc.sync.dma_start(e[:], epsT[:, off * KT : off * KT + KT * w])
                    eps_tiles[b] = e

            wt = {}

            QW = KT * 256  # one dout-quarter of a weight image

            def fetch_weight(name, engines):
                """Fetch a weight image quarter-by-quarter; engines[i] is the
                DMA queue for quarter i (quarters land progressively, in the
                order the matmul m-pair loop consumes them)."""
                w = wpool.tile([128, KT * 1024], BF16,
                               tag=f"w_{name}", name=f"wt_{name}")
                for i, eng in enumerate(engines):
                    eng.dma_start(w[:, i * QW : (i + 1) * QW],
                                  wdram[name][:, i * QW : (i + 1) * QW])
                wt[name] = w

            # PE warm-up: engine init holds DMA queues until ~8us and the
            # first weight quarter lands ~14us in, during which an idle PE
            # would HAM-throttle to 1.2GHz.  Bridge the whole window with
            # dependency-free dummy matmuls (first ~16 run cold, the rest
            # at 2.4GHz) so the real stream starts warm the moment its
            # inputs land.  They cycle the DECODER psum ring (idle until
            # ~t=50us) so the encoder's first psum never waits behind them.
            warm = io2.tile([128, 256], BF16, tag="warm")
            nc.vector.memset(warm[:], 0.0)
            for _ in range(48):
                ps_w = psum_d.tile([128, 512], F32, tag="ps")
                nc.tensor.matmul(ps_w[:, :256], warm[:, :128], warm[:],
                                 start=True, stop=True)

            # DMA schedule: weight quarters are interleaved across the three
            # DMA-capable queues (sync/gpsimd/scalar) in the order the
            # matmul stream consumes them -- per-queue bandwidth is only
            # ~110-150GB/s, so a single queue cannot keep up.  wv1 rides
            # the sync queue right behind x0 (eps/x of later blocks are
            # not needed until much later).
            gp, sc = nc.gpsimd, nc.scalar
            fetch_block(0, x_only=True)
            fetch_weight("wm1", [gp, sc, gp, sc])
            fetch_weight("wv1", [nc.sync] * 4)
            fetch_block(0)
            fetch_block(1)
            fetch_weight("wm2", [gp, gp, gp, gp])
            fetch_weight("wv2", [sc, sc, sc, sc])
            fetch_weight("we1", [gp, gp, gp, gp])
            fetch_weight("we2", [sc, sc, sc, sc])

            def layer(w, rhs_fn, tw, out_cb, pool=None, first_after=None):
                """One 1024->1024 matmul layer on a [128, tw]-sliced bf16 rhs.

                rhs_fn(k) yields the [128, tw] rhs slice for k-tile k.
                out_cb(mp, ps) consumes the [128, 2*tw] f32 psum of m-pair mp
                (psum tiles are always allocated [128, 512]; only the first
                2*tw columns are written).  first_after: ordering hint --
                schedule this layer's first matmul after that instruction.
                """
                pool = pool or psum_e
                mm = None
                for mp in range(4):
                    ps = pool.tile([128, 512], F32, tag="ps")
                    for half in range(2):
                        m = 2 * mp + half
                        wo = (m // 2) * (KT * 256) + (m % 2) * 128
                        for k in range(KT):
                            mm = nc.tensor.matmul(
                                ps[:, half * tw : (half + 1) * tw],
                                w[:, wo + k * 256 : wo + k * 256 + 128],
                                rhs_fn(k),
                                start=(k == 0),
                                stop=(k == KT - 1),
                            )
                            if first_after is not None:
                                tile.add_dep_helper(
                                    mm.ins, first_after.ins, sync=False,
                                    reason="decoder pipelined behind next block")
                                first_after = None
                    out_cb(mp, ps)
                return mm

            def flat_rhs(t, tw):
                return lambda k: t[:, k * tw : (k + 1) * tw]

            def quartered_rhs(parts, tw):
                # parts[q] holds k-tiles 2q and 2q+1 side by side
                return lambda k: parts[k // 2][:, (k % 2) * tw :
                                               (k % 2 + 1) * tw]

            def leaky_to(dst_fn, tw):
                def cb(mp, ps):
                    # leaky(x) = max(x, 0.01x); DVE can read PSUM only
                    # once per op, so stage 0.01x in SBUF first.
                    lk = io2.tile([128, 512], F32, tag="lk")
                    nc.vector.tensor_scalar_mul(
                        lk[:, : 2 * tw], ps[:, : 2 * tw], SLOPE)
                    nc.vector.tensor_tensor(
                        dst_fn(mp), lk[:, : 2 * tw], ps[:, : 2 * tw], max_)
                return cb

            def enc_block(b):
                """Encoder + sampling for block b; returns the bf16 h tiles
                (one [128, 2*tw] tile per feature-quarter, so the decoder's
                first k-tiles can start before the whole sampling chain
                drains -- matters for the final block's decoder)."""
                off, tw = BLOCKS[b]
                x, epst = xt_tiles.pop(b), eps_tiles.pop(b)
                fetch_block(b + 1)

                h1m = mid.tile([128, KT * tw], BF16, tag="h1m")
                l1m_last = layer(
                    wt["wm1"], flat_rhs(x, tw), tw,
                    leaky_to(lambda mp: h1m[:, 2 * mp * tw : (2 * mp + 2) * tw],
                             tw))
                h1v = mid.tile([128, KT * tw], BF16, tag="h1v")
                layer(wt["wv1"], flat_rhs(x, tw), tw,
                      leaky_to(lambda mp: h1v[:, 2 * mp * tw : (2 * mp + 2) * tw],
                               tw))

                mu_f = io.tile([128, KT * tw], F32, tag="mu_f")

                def mu_cb(mp, ps):
                    nc.scalar.activation(
                        mu_f[:, 2 * mp * tw : (2 * mp + 2) * tw],
                        ps[:, : 2 * tw], Copy)

                layer(wt["wm2"], flat_rhs(h1m, tw), tw, mu_cb)
                nc.sync.dma_start(outs["muT"][:, off * KT : off * KT + KT * tw],
                                  mu_f[:])

                std_f = mid.tile([128, KT * tw], F32, tag="std_f")
                tmp_f = mid.tile([128, KT * tw], F32, tag="tmp_f")
                h_f = io.tile([128, KT * tw], F32, tag="h_f")
                hbq = [io2.tile([128, 512], BF16, tag=f"hb{q}",
                                name=f"hb{q}") for q in range(4)]

                def lv_cb(mp, ps):
                    sl = slice(2 * mp * tw, (2 * mp + 2) * tw)
                    # lv staged per m-pair so the DMA drains early and the
                    # full-block f32 tile is saved.
                    lv_s = iost.tile([128, 512], F32, tag="lv_s")
                    nc.scalar.activation(lv_s[:, : 2 * tw], ps[:, : 2 * tw],
                                         Copy)
                    nc.sync.dma_start(
                        outs["lvT"][:, off * KT + 2 * mp * tw :
                                    off * KT + (2 * mp + 2) * tw],
                        lv_s[:, : 2 * tw])
                    nc.scalar.activation(std_f[:, sl], ps[:, : 2 * tw],
                                         Exp, scale=0.5)
                    # h = eps*std + mu, per m-pair so it pipelines
                    nc.vector.tensor_tensor(
                        tmp_f[:, sl], epst[:, sl], std_f[:, sl], mult)
                    nc.vector.tensor_tensor(
                        h_f[:, sl], tmp_f[:, sl], mu_f[:, sl], add)
                    nc.vector.tensor_tensor(
                        hbq[mp][:, : 2 * tw], tmp_f[:, sl], mu_f[:, sl], add)

                layer(wt["wv2"], flat_rhs(h1v, tw), tw, lv_cb)
                nc.sync.dma_start(outs["hT"][:, off * KT : off * KT + KT * tw],
                                  h_f[:])
                return hbq, l1m_last

            def dec_block(b, hbq, after=None):
                """Decoder (this core's main expert) for block b."""
                off, tw = BLOCKS[b]
                d1q = [mid.tile([128, 512], BF16, tag=f"d1{q}",
                                name=f"d1{q}") for q in range(4)]
                layer(wt["we1"], quartered_rhs(hbq, tw), tw,
                      leaky_to(lambda mp: d1q[mp][:, : 2 * tw], tw),
                      pool=psum_d, first_after=after)

                def rec_cb(mp, ps):
                    rec_s = iost.tile([128, 512], F32, tag="rec_s")
                    nc.scalar.activation(rec_s[:, : 2 * tw], ps[:, : 2 * tw],
                                         Copy)
                    # per-m-pair output DMA so the tail drains early
                    nc.sync.dma_start(
                        outs["recT"][:, off * KT + 2 * mp * tw :
                                     off * KT + (2 * mp + 2) * tw],
                        rec_s[:, : 2 * tw])

                layer(wt["we2"], quartered_rhs(d1q, tw), tw, rec_cb,
                      pool=psum_d)

            # Software-pipeline the decoder one block behind the encoder:
            # while block b's sampling chain (ACT exp + DVE fma) drains,
            # the PE is busy on block b-1's decoder -- no PE idle at block
            # boundaries (which would also re-throttle the HAM clock).
            prev = None
            for b in range(NBLOCKS):
                h_b, l1m_last = enc_block(b)
                if prev is not None:
                    dec_block(b - 1, prev, after=l1m_last)
                prev = h_b
            dec_block(NBLOCKS - 1, prev)

    nc.compile()
    return nc


def _get_program() -> "bacc.Bacc":
    if "v2" not in _program_cache:
        _program_cache["v2"] = build_program()
    return _program_cache["v2"]


def _to_sbuf_image(arrT, width):
    """[1024, width] feature-major -> [128, KT*width] flat image, blockwise."""
    out = np.empty((128, KT * arrT.shape[1]), dtype=arrT.dtype)
    for off in range(0, arrT.shape[1], T):
        w = min(T, arrT.shape[1] - off)
        seg = arrT[:, off:off + w].reshape(KT, 128, w).transpose(1, 0, 2)
        out[:, off * KT : off * KT + KT * w] = seg.reshape(128, KT * w)
    return out


def _from_sbuf_image(img, width):
    """[128, KT*width] flat SBUF image -> [1024, width] feature-major."""
    out = np.empty((1024, width), dtype=img.dtype)
    for off in range(0, width, T):
        w = min(T, width - off)
        seg = img[:, off * KT : off * KT + KT * w].reshape(128, KT, w)
        out[:, off:off + w] = seg.transpose(1, 0, 2).reshape(1024, w)
    return out


def _weight_image(W):
    """[1024 din, 1024 dout] -> [128, KT*1024] flat lhsT image.

    dout-quarter-major: flat = (m//256)*(KT*256) + k*256 + (m%256), so each
    dout quarter is one contiguous per-partition range (the head can load
    wm1 quarter-by-quarter and start matmuls at 1/4 the critical DMA bytes).
    """
    img = W.reshape(KT, 128, 4, 256).transpose(1, 2, 0, 3)
    return np.ascontiguousarray(img.reshape(128, KT * 1024))


def _kernel_numpy(inputs):
    """Exact f32 fallback (used only if an assumption is violated)."""
    d = {k: np.asarray(v) for k, v in inputs.items()}
    leaky = lambda v: np.where(v > 0, v, np.float32(SLOPE) * v)
    mu = leaky(d["data"] @ d["Wm1"] + d["bm1"]) @ d["Wm2"] + d["bm2"]
    lv = leaky(d["data"] @ d["Wv1"] + d["bv1"]) @ d["Wv2"] + d["bv2"]
    h = d["eps"] * np.exp(0.5 * lv) + mu
    s = np.asarray(d["s"]).astype(np.int64)
    rec = np.empty_like(d["data"])
    for e in range(d["We1"].shape[0]):
        m = s == e
        rec[m] = leaky(h[m] @ d["We1"][e] + d["be1"][e]) @ d["We2"][e] + d["be2"][e]
    return rec, mu, lv, h


def _partition_tokens(s):
    """Cut the expert-sorted token order into NCORES chunks of C tokens.

    Every chunk is decoded on-device with its MAJORITY expert's weights;
    the few minority ("rest") tokens at the chunk edges get their decoder
    recomputed on the host (0.6% of tokens for balanced random s) from the
    device-computed h_sample.  Returns per-core (ids, main_expert) where
    ids is ordered main-tokens-first.
    """
    order = np.argsort(s, kind="stable")
    cores = []
    for c in range(NCORES):
        chunk = order[c * C : (c + 1) * C]
        experts = s[chunk]
        uniq, counts = np.unique(experts, return_counts=True)
        main_e = int(uniq[np.argmax(counts)])
        is_main = experts == main_e
        ids = np.concatenate([chunk[is_main], chunk[~is_main]])
        cores.append((ids, main_e))
    return cores


def kernel(**inputs) -> tuple:
    data = np.ascontiguousarray(np.asarray(inputs["data"], dtype=np.float32))
    eps = np.ascontiguousarray(np.asarray(inputs["eps"], dtype=np.float32))
    s = np.asarray(inputs["s"]).astype(np.int64)
    # The device kernel folds the (structurally zero) biases away; any
    # violated assumption falls back to an exact host computation.
    nonzero_bias = any(
        np.abs(np.asarray(inputs[b])).max() != 0.0
        for b in ("bm1", "bm2", "bv1", "bv2", "be1", "be2"))
    if nonzero_bias or data.shape != (N, DX) or s.shape != (N,):
        return _kernel_numpy(inputs)
    cores = _partition_tokens(s)

    nc = _get_program()
    bf = ml_dtypes.bfloat16

    wimg = {
        "wm1": _weight_image(np.asarray(inputs["Wm1"], np.float32)).astype(bf),
        "wm2": _weight_image(np.asarray(inputs["Wm2"], np.float32)).astype(bf),
        "wv1": _weight_image(np.asarray(inputs["Wv1"], np.float32)).astype(bf),
        "wv2": _weight_image(np.asarray(inputs["Wv2"], np.float32)).astype(bf),
    }
    We1 = np.asarray(inputs["We1"], np.float32)
    We2 = np.asarray(inputs["We2"], np.float32)
    dec_img = {}

    def dec_images(e):
        if e not in dec_img:
            dec_img[e] = (_weight_image(We1[e]).astype(bf),
                          _weight_image(We2[e]).astype(bf))
        return dec_img[e]

    dataT = data.T
    epsT = eps.T

    in_maps = []
    for ids, main_e in cores:
        we1, we2 = dec_images(main_e)
        in_maps.append({
            "xT": _to_sbuf_image(
                np.ascontiguousarray(dataT[:, ids]).astype(bf), C),
            "epsT": _to_sbuf_image(
                np.ascontiguousarray(epsT[:, ids]).astype(bf), C),
            "wm1": wimg["wm1"], "wm2": wimg["wm2"],
            "wv1": wimg["wv1"], "wv2": wimg["wv2"],
            "we1": we1, "we2": we2,
        })

    global LAST_RESULTS
    _ensure_ntff_hook()
    res = run_bass_kernel_spmd(nc, in_maps, list(range(NCORES)))
    LAST_RESULTS = res

    mu = np.empty((N, DH), np.float32)
    lv = np.empty((N, DH), np.float32)
    h = np.empty((N, DH), np.float32)
    rec = np.empty((N, DX), np.float32)
    for c, (ids, main_e) in enumerate(cores):
        r = res.results[c]
        mu[ids] = _from_sbuf_image(r["muT"], C).T
        lv[ids] = _from_sbuf_image(r["lvT"], C).T
        h[ids] = _from_sbuf_image(r["hT"], C).T
        rec[ids] = _from_sbuf_image(r["recT"], C).T

    # Host-side decoder for the minority tokens of each chunk (<=0.6% of
    # tokens): their on-device decode used the majority expert's weights.
    leaky = lambda v: np.where(v > 0, v, np.float32(SLOPE) * v)
    rest = np.concatenate([ids[s[ids] != main_e] for ids, main_e in cores])
    if rest.size:
        for e in np.unique(s[rest]):
            m = rest[s[rest] == e]
            rec[m] = leaky(h[m] @ We1[e]) @ We2[e]
    return rec, mu, lv, h


# revision 21
# speedup vs baseline: 1.0321x; 1.0228x over previous
"""Trainium2 Bass kernel for nn_Causal_model_vae (MoE-routed VAE).

Reference computation (N=16384 tokens, DX=DH=1024, S=8 experts):
    mu_h     = leaky(data @ Wm1 + bm1) @ Wm2 + bm2
    logvar_h = leaky(data @ Wv1 + bv1) @ Wv2 + bv2
    h_sample = eps * exp(0.5*logvar_h) + mu_h
    reconstruct[n] = (leaky(h_sample @ We1[s_n] + be1[s_n]) @ We2[s_n] + be2[s_n])
returns (reconstruct, mu_h, logvar_h, h_sample).

Strategy: perfectly load-balanced expert-parallelism.  The routing ids `s`
are known on the host, so the host sorts tokens by expert and cuts the
sorted order into 8 chunks of exactly N/8 = 2048 tokens -- every core does
identical work (the bf16 PE roofline is 6 matmul layers x 2048 tokens).
A chunk's minority tokens (those not of its majority expert -- <=0.6% for
balanced random s) are decoded on-device with the WRONG (majority) weights
to keep the SPMD program uniform; the host recomputes just their decoder
from the device-computed h_sample and overwrites those rows.

On-chip layout: everything is feature-major [feature, token] so chained
matmuls need no transposes; matmul operands are bf16 (f32 PSUM
accumulation), the sampling chain and all outputs are f32.  All DRAM
tensors use the flat SBUF-image layout [128, KT*width] so every DMA is
fully contiguous per partition.  Weight fetches ride the gpsimd queue in
parallel with the sync queue's x/eps fetches so the first matmul starts
~4us earlier; the aux We2 image reuses wm1's SBUF buffer (dead after the
last encoder block) to stay inside SBUF.

The decoder is software-pipelined one token-block behind the encoder so
the PE never idles while the sampling chain (ACT exp + DVE fma) drains.

Biases are structurally zero in this problem's setup_inputs(); the kernel
asserts that and skips them on-device.  Any violated structural
assumption falls back to an exact host computation.
"""

import contextlib
import ctypes
import os
import sys
import types

import numpy as np
import ml_dtypes

import concourse.bacc as bacc
import concourse.bass as bass
import concourse.mybir as mybir
import concourse.tile as tile
from concourse.bass_utils import run_bass_kernel_spmd

N, DX, DH, S = 16384, 1024, 1024, 8
KT = DH // 128    # 8 k-tiles (DX == DH == 1024)
SLOPE = 0.01
NCORES = 8
T = 256           # token block width (matmul moving dim)
C = N // NCORES   # 2048 tokens per core, exactly balanced

BF16 = mybir.dt.bfloat16
F32 = mybir.dt.float32

LAST_RESULTS = None  # BassKernelResults of the most recent run (for profiling)

_program_cache: dict[str, "bacc.Bacc"] = {}


def _ensure_ntff_hook():
    """bass_utils imports antenv.axon_hooks when tracing under axon; some
    images lack that module.  Install a ctypes-based equivalent if so."""
    try:
        import antenv.axon_hooks  # noqa: F401
        return
    except ImportError:
        pass
    try:
        import antenv

        so_path = "/opt/axon/libaxon_pjrt.so"
        if not os.path.exists(so_path):
            return
        lib = ctypes.CDLL(so_path)
        if not hasattr(lib, "axon_start_nrt_profile"):
            return
        lib.axon_start_nrt_profile.argtypes = [
            ctypes.POINTER(ctypes.c_int64), ctypes.c_size_t]
        lib.axon_start_nrt_profile.restype = ctypes.c_int64
        lib.axon_stop_nrt_profile.argtypes = [ctypes.c_char_p]
        lib.axon_stop_nrt_profile.restype = ctypes.c_int64

        @contextlib.contextmanager
        def _hook(output_dir, device_ids):
            import jax

            jax.devices()
            if device_ids:
                ids = (ctypes.c_int64 * len(device_ids))(*device_ids)
                rc = lib.axon_start_nrt_profile(ids, len(device_ids))
            else:
                rc = lib.axon_start_nrt_profile(None, 0)
            if rc != 0:
                raise RuntimeError(f"axon_start_nrt_profile rc={rc}")
            try:
                yield
            finally:
                n = lib.axon_stop_nrt_profile(str(output_dir).encode())
                print(f"ntff profile: {n} file(s) -> {output_dir}")

        m = types.ModuleType("antenv.axon_hooks")
        m.get_axon_ntff_profile_hook = lambda: _hook
        m.set_axon_ntff_profile_hook = lambda h: None
        sys.modules["antenv.axon_hooks"] = m
        antenv.axon_hooks = m
    except Exception:
        pass


BLOCKS = [(i * T, T) for i in range(C // T)]   # 8 x 256
NBLOCKS = len(BLOCKS)


def build_program() -> "bacc.Bacc":
    nc = bacc.Bacc("TRN2", target_bir_lowering=False, debug=False,
                   num_devices=NCORES)

    xT = nc.dram_tensor("xT", [128, KT * C], BF16, kind="ExternalInput").ap()
    epsT = nc.dram_tensor("epsT", [128, KT * C], BF16,
                          kind="ExternalInput").ap()
    wnames = ["wm1", "wv1", "wm2", "wv2", "we1", "we2"]
    wdram = {n: nc.dram_tensor(n, [128, KT * 1024], BF16,
                               kind="ExternalInput").ap() for n in wnames}
    # Outputs are bf16 (host upcasts): ~1e-3 extra rel err, half the DMA
    # bytes -- the three DMA queues cap at ~105-150GB/s each and f32
    # outputs saturate them (the kernel then stalls at the end waiting for
    # the output-queue backlog to drain).
    outs = {n: nc.dram_tensor(n, [128, KT * C], BF16,
                              kind="ExternalOutput").ap()
            for n in ["muT", "lvT", "hT", "recT"]}

    Exp = mybir.ActivationFunctionType.Exp
    Copy = mybir.ActivationFunctionType.Copy
    mult = mybir.AluOpType.mult
    max_ = mybir.AluOpType.max
    add = mybir.AluOpType.add

    with tile.TileContext(nc) as tc:
        with (
            tc.tile_pool(name="wpool", bufs=1) as wpool,
            tc.tile_pool(name="io2", bufs=2) as io2,
            tc.tile_pool(name="io", bufs=1) as io,
            tc.tile_pool(name="iost", bufs=2) as iost,
            tc.tile_pool(name="mid", bufs=1) as mid,
            # Separate PSUM pools so the (one-block-delayed) decoder's
            # slot-recycling waits never reference encoder matmul progress.
            tc.tile_pool(name="psum_e", bufs=5,
                         space=bass.MemorySpace.PSUM) as psum_e,
            tc.tile_pool(name="psum_d", bufs=3,
                         space=bass.MemorySpace.PSUM) as psum_d,
        ):
            xt_tiles = {}
            eps_tiles = {}

            def fetch_block(b, x_only=False):
                if b >= NBLOCKS:
                    return
                off, w = BLOCKS[b]
                if b not in xt_tiles:
                    x = io2.tile([128, KT * w], BF16, tag="x")
                    nc.sync.dma_start(x[:], xT[:, off * KT : off * KT + KT * w])
                    xt_tiles[b] = x
                if not x_only and b not in eps_tiles:
                    e = io2.tile([128, KT * w], BF16, tag="eps")
                    nc.sync.dma_start(e[:], epsT[:, off * KT : off * KT + KT * w])
                    eps_tiles[b] = e

            wt = {}

            QW = KT * 256  # one dout-quarter of a weight image

            def fetch_weight(name, engines):
                """Fetch a weight image as FOUR SEPARATE quarter tiles;
                engines[i] is the DMA queue for quarter i.  Tile tracks
                dependencies per tile, so separate tiles are what lets the
                matmul m-pair loop start on quarter 0 while later quarters
                are still in flight."""
                qs = []
                for i, eng in enumerate(engines):
                    w = wpool.tile([128, QW], BF16,
                                   tag=f"w_{name}q{i}", name=f"wt_{name}q{i}")
                    eng.dma_start(w[:], wdram[name][:, i * QW : (i + 1) * QW])
                    qs.append(w)
                wt[name] = qs

            # PE warm-up: engine init holds DMA queues until ~8us and the
            # first weight quarter lands ~14us in, during which an idle PE
            # would HAM-throttle to 1.2GHz.  Bridge the whole window with
            # dependency-free dummy matmuls (first ~16 run cold, the rest
            # at 2.4GHz) so the real stream starts warm the moment its
            # inputs land.  They cycle the DECODER psum ring (idle until
            # ~t=50us) so the encoder's first psum never waits behind them.
            warm = io2.tile([128, 256], BF16, tag="warm")
            nc.vector.memset(warm[:], 0.0)
            for _ in range(48):
                ps_w = psum_d.tile([128, 512], F32, tag="ps")
                nc.tensor.matmul(ps_w[:, :256], warm[:, :128], warm[:],
                                 start=True, stop=True)

            # DMA schedule: weight quarters are interleaved across the three
            # DMA-capable queues (sync/gpsimd/scalar) in the order the
            # matmul stream consumes them -- per-queue bandwidth is only
            # ~110-150GB/s, so a single queue cannot keep up.  wv1 rides
            # the sync queue right behind x0 (eps/x of later blocks are
            # not needed until much later).
            gp, sc = nc.gpsimd, nc.scalar
            fetch_block(0, x_only=True)
            fetch_weight("wm1", [gp, sc, gp, sc])
            fetch_weight("wv1", [nc.sync] * 4)
            fetch_block(0)
            fetch_block(1)
            fetch_weight("wm2", [gp, gp, gp, gp])
            fetch_weight("wv2", [sc, sc, sc, sc])
            fetch_weight("we1", [gp, gp, gp, gp])
            fetch_weight("we2", [sc, sc, sc, sc])

            def layer(w, rhs_fn, tw, out_cb, pool=None):
                """One 1024->1024 matmul layer on a [128, tw]-sliced bf16 rhs.

                w is a list of 4 dout-quarter weight tiles [128, KT*256].
                rhs_fn(k) yields the [128, tw] rhs slice for k-tile k.
                out_cb(mp, ps) consumes the [128, 2*tw] f32 psum of m-pair mp
                (psum tiles are always allocated [128, 512]; only the first
                2*tw columns are written).
                """
                pool = pool or psum_e
                mm = None
                for mp in range(4):
                    ps = pool.tile([128, 512], F32, tag="ps")
                    for half in range(2):
                        wo = half * 128
                        for k in range(KT):
                            mm = nc.tensor.matmul(
                                ps[:, half * tw : (half + 1) * tw],
                                w[mp][:, wo + k * 256 : wo + k * 256 + 128],
                                rhs_fn(k),
                                start=(k == 0),
                                stop=(k == KT - 1),
                            )
                    out_cb(mp, ps)
                return mm

            def flat_rhs(t, tw):
                return lambda k: t[:, k * tw : (k + 1) * tw]

            def quartered_rhs(parts, tw):
                # parts[q] holds k-tiles 2q and 2q+1 side by side
                return lambda k: parts[k // 2][:, (k % 2) * tw :
                                               (k % 2 + 1) * tw]

            def leaky_to(dst_fn, tw):
                def cb(mp, ps):
                    # leaky(x) = max(x, 0.01x); DVE can read PSUM only
                    # once per op, so stage 0.01x in SBUF first.
                    lk = io2.tile([128, 512], F32, tag="lk")
                    nc.vector.tensor_scalar_mul(
                        lk[:, : 2 * tw], ps[:, : 2 * tw], SLOPE)
                    nc.vector.tensor_tensor(
                        dst_fn(mp), lk[:, : 2 * tw], ps[:, : 2 * tw], max_)
                return cb

            def enc_block(b):
                """Encoder + sampling for block b; returns the bf16 h tiles
                (one [128, 2*tw] tile per feature-quarter, so the decoder's
                first k-tiles can start before the whole sampling chain
                drains -- matters for the final block's decoder)."""
                off, tw = BLOCKS[b]
                x, epst = xt_tiles.pop(b), eps_tiles.pop(b)
                fetch_block(b + 1)

                h1m = mid.tile([128, KT * tw], BF16, tag="h1m")
                layer(wt["wm1"], flat_rhs(x, tw), tw,
                      leaky_to(lambda mp: h1m[:, 2 * mp * tw : (2 * mp + 2) * tw],
                               tw))
                h1v = mid.tile([128, KT * tw], BF16, tag="h1v")
                layer(wt["wv1"], flat_rhs(x, tw), tw,
                      leaky_to(lambda mp: h1v[:, 2 * mp * tw : (2 * mp + 2) * tw],
                               tw))

                mu_f = io.tile([128, KT * tw], BF16, tag="mu_f")

                def mu_cb(mp, ps):
                    nc.scalar.activation(
                        mu_f[:, 2 * mp * tw : (2 * mp + 2) * tw],
                        ps[:, : 2 * tw], Copy)

                layer(wt["wm2"], flat_rhs(h1m, tw), tw, mu_cb)
                nc.gpsimd.dma_start(
                    outs["muT"][:, off * KT : off * KT + KT * tw], mu_f[:])

                std_f = mid.tile([128, KT * tw], F32, tag="std_f")
                tmp_f = mid.tile([128, KT * tw], F32, tag="tmp_f")
                hbq = [io2.tile([128, 512], BF16, tag=f"hb{q}",
                                name=f"hb{q}") for q in range(4)]

                def lv_cb(mp, ps):
                    sl = slice(2 * mp * tw, (2 * mp + 2) * tw)
                    # lv staged per m-pair so the DMA drains early and the
                    # full-block f32 tile is saved.
                    lv_s = iost.tile([128, 512], BF16, tag="lv_s")
                    nc.scalar.activation(lv_s[:, : 2 * tw], ps[:, : 2 * tw],
                                         Copy)
                    nc.scalar.dma_start(
                        outs["lvT"][:, off * KT + 2 * mp * tw :
                                    off * KT + (2 * mp + 2) * tw],
                        lv_s[:, : 2 * tw])
                    nc.scalar.activation(std_f[:, sl], ps[:, : 2 * tw],
                                         Exp, scale=0.5)
                    # h = eps*std + mu, per m-pair; the bf16 quarter tile is
                    # both the decoder rhs and the hT output (one DVE op).
                    nc.vector.tensor_tensor(
                        tmp_f[:, sl], epst[:, sl], std_f[:, sl], mult)
                    nc.vector.tensor_tensor(
                        hbq[mp][:, : 2 * tw], tmp_f[:, sl], mu_f[:, sl], add)
                    nc.gpsimd.dma_start(
                        outs["hT"][:, off * KT + 2 * mp * tw :
                                   off * KT + (2 * mp + 2) * tw],
                        hbq[mp][:, : 2 * tw])

                layer(wt["wv2"], flat_rhs(h1v, tw), tw, lv_cb)
                return hbq

            def dec_block(b, hbq):
                """Decoder (this core's main expert) for block b."""
                off, tw = BLOCKS[b]
                d1q = [mid.tile([128, 512], BF16, tag=f"d1{q}",
                                name=f"d1{q}") for q in range(4)]
                layer(wt["we1"], quartered_rhs(hbq, tw), tw,
                      leaky_to(lambda mp: d1q[mp][:, : 2 * tw], tw),
                      pool=psum_d)

                def rec_cb(mp, ps):
                    rec_s = iost.tile([128, 512], BF16, tag="rec_s")
                    nc.scalar.activation(rec_s[:, : 2 * tw], ps[:, : 2 * tw],
                                         Copy)
                    # per-m-pair output DMA so the tail drains early
                    nc.sync.dma_start(
                        outs["recT"][:, off * KT + 2 * mp * tw :
                                     off * KT + (2 * mp + 2) * tw],
                        rec_s[:, : 2 * tw])

                layer(wt["we2"], quartered_rhs(d1q, tw), tw, rec_cb,
                      pool=psum_d)

            # Software-pipeline the decoder one block behind the encoder:
            # while block b's sampling chain (ACT exp + DVE fma) drains,
            # the PE is busy on block b-1's decoder -- no PE idle at block
            # boundaries (which would also re-throttle the HAM clock).
            prev = None
            for b in range(NBLOCKS):
                h_b = enc_block(b)
                if prev is not None:
                    dec_block(b - 1, prev)
                prev = h_b
            dec_block(NBLOCKS - 1, prev)

    nc.compile()
    return nc


def _get_program() -> "bacc.Bacc":
    if "v2" not in _program_cache:
        _program_cache["v2"] = build_program()
    return _program_cache["v2"]


def _to_sbuf_image(arrT, width):
    """[1024, width] feature-major -> [128, KT*width] flat image, blockwise."""
    out = np.empty((128, KT * arrT.shape[1]), dtype=arrT.dtype)
    for off in range(0, arrT.shape[1], T):
        w = min(T, arrT.shape[1] - off)
        seg = arrT[:, off:off + w].reshape(KT, 128, w).transpose(1, 0, 2)
        out[:, off * KT : off * KT + KT * w] = seg.reshape(128, KT * w)
    return out


def _from_sbuf_image(img, width):
    """[128, KT*width] flat SBUF image -> [1024, width] feature-major."""
    out = np.empty((1024, width), dtype=img.dtype)
    for off in range(0, width, T):
        w = min(T, width - off)
        seg = img[:, off * KT : off * KT + KT * w].reshape(128, KT, w)
        out[:, off:off + w] = seg.transpose(1, 0, 2).reshape(1024, w)
    return out


def _weight_image(W):
    """[1024 din, 1024 dout] -> [128, KT*1024] flat lhsT image.

    dout-quarter-major: flat = (m//256)*(KT*256) + k*256 + (m%256), so each
    dout quarter is one contiguous per-partition range (the head can load
    wm1 quarter-by-quarter and start matmuls at 1/4 the critical DMA bytes).
    """
    img = W.reshape(KT, 128, 4, 256).transpose(1, 2, 0, 3)
    return np.ascontiguousarray(img.reshape(128, KT * 1024))


def _kernel_numpy(inputs):
    """Exact f32 fallback (used only if an assumption is violated)."""
    d = {k: np.asarray(v) for k, v in inputs.items()}
    leaky = lambda v: np.where(v > 0, v, np.float32(SLOPE) * v)
    mu = leaky(d["data"] @ d["Wm1"] + d["bm1"]) @ d["Wm2"] + d["bm2"]
    lv = leaky(d["data"] @ d["Wv1"] + d["bv1"]) @ d["Wv2"] + d["bv2"]
    h = d["eps"] * np.exp(0.5 * lv) + mu
    s = np.asarray(d["s"]).astype(np.int64)
    rec = np.empty_like(d["data"])
    for e in range(d["We1"].shape[0]):
        m = s == e
        rec[m] = leaky(h[m] @ d["We1"][e] + d["be1"][e]) @ d["We2"][e] + d["be2"][e]
    return rec, mu, lv, h


def _partition_tokens(s):
    """Cut the expert-sorted token order into NCORES chunks of C tokens.

    Every chunk is decoded on-device with its MAJORITY expert's weights;
    the few minority ("rest") tokens at the chunk edges get their decoder
    recomputed on the host (0.6% of tokens for balanced random s) from the
    device-computed h_sample.  Returns per-core (ids, main_expert) where
    ids is ordered main-tokens-first.
    """
    order = np.argsort(s, kind="stable")
    cores = []
    for c in range(NCORES):
        chunk = order[c * C : (c + 1) * C]
        experts = s[chunk]
        uniq, counts = np.unique(experts, return_counts=True)
        main_e = int(uniq[np.argmax(counts)])
        is_main = experts == main_e
        ids = np.concatenate([chunk[is_main], chunk[~is_main]])
        cores.append((ids, main_e))
    return cores


def kernel(**inputs) -> tuple:
    data = np.ascontiguousarray(np.asarray(inputs["data"], dtype=np.float32))
    eps = np.ascontiguousarray(np.asarray(inputs["eps"], dtype=np.float32))
    s = np.asarray(inputs["s"]).astype(np.int64)
    # The device kernel folds the (structurally zero) biases away; any
    # violated assumption falls back to an exact host computation.
    nonzero_bias = any(
        np.abs(np.asarray(inputs[b])).max() != 0.0
        for b in ("bm1", "bm2", "bv1", "bv2", "be1", "be2"))
    if nonzero_bias or data.shape != (N, DX) or s.shape != (N,):
        return _kernel_numpy(inputs)
    cores = _partition_tokens(s)

    nc = _get_program()
    bf = ml_dtypes.bfloat16

    wimg = {
        "wm1": _weight_image(np.asarray(inputs["Wm1"], np.float32)).astype(bf),
        "wm2": _weight_image(np.asarray(inputs["Wm2"], np.float32)).astype(bf),
        "wv1": _weight_image(np.asarray(inputs["Wv1"], np.float32)).astype(bf),
        "wv2": _weight_image(np.asarray(inputs["Wv2"], np.float32)).astype(bf),
    }
    We1 = np.asarray(inputs["We1"], np.float32)
    We2 = np.asarray(inputs["We2"], np.float32)
    dec_img = {}

    def dec_images(e):
        if e not in dec_img:
            dec_img[e] = (_weight_image(We1[e]).astype(bf),
                          _weight_image(We2[e]).astype(bf))
        return dec_img[e]

    dataT = data.T
    epsT = eps.T

    in_maps = []
    for ids, main_e in cores:
        we1, we2 = dec_images(main_e)
        in_maps.append({
            "xT": _to_sbuf_image(
                np.ascontiguousarray(dataT[:, ids]).astype(bf), C),
            "epsT": _to_sbuf_image(
                np.ascontiguousarray(epsT[:, ids]).astype(bf), C),
            "wm1": wimg["wm1"], "wm2": wimg["wm2"],
            "wv1": wimg["wv1"], "wv2": wimg["wv2"],
            "we1": we1, "we2": we2,
        })

    global LAST_RESULTS
    _ensure_ntff_hook()
    res = run_bass_kernel_spmd(nc, in_maps, list(range(NCORES)))
    LAST_RESULTS = res

    mu = np.empty((N, DH), np.float32)
    lv = np.empty((N, DH), np.float32)
    h = np.empty((N, DH), np.float32)
    rec = np.empty((N, DX), np.float32)
    for c, (ids, main_e) in enumerate(cores):
        r = res.results[c]
        mu[ids] = _from_sbuf_image(r["muT"], C).T
        lv[ids] = _from_sbuf_image(r["lvT"], C).T
        h[ids] = _from_sbuf_image(r["hT"], C).T
        rec[ids] = _from_sbuf_image(r["recT"], C).T

    # Host-side decoder for the minority tokens of each chunk (<=0.6% of
    # tokens): their on-device decode used the majority expert's weights.
    leaky = lambda v: np.where(v > 0, v, np.float32(SLOPE) * v)
    rest = np.concatenate([ids[s[ids] != main_e] for ids, main_e in cores])
    if rest.size:
        for e in np.unique(s[rest]):
            m = rest[s[rest] == e]
            rec[m] = leaky(h[m] @ We1[e]) @ We2[e]
    return rec, mu, lv, h
